# revision 1
# baseline (speedup 1.0000x reference)
"""Trainium2 Bass kernel for the DPAG pairwise-attention + MLP module.

Data-parallel over batch: B=8 batch elements, one per NeuronCore.
Each core computes its full batch row end-to-end on device; the host
only slices inputs per core and stacks the (2,)-outputs back to (8,2).

Math (per batch element, fused — the (Nd,Np,D) intermediate is never
materialized):
    U = concat([smi @ w_att + b_att, gat], 0)          # (145, 64)
    V = pro @ w_att + b_att                            # (1000, 64)
    S[i] = sum_j relu(U[i] + V[j])                     # (145, 64)
    T[j] = sum_i relu(U[i] + V[j])                     # (1000, 64)
    g1 = sigmoid((S/1000) @ w_att + b_att)             # (145, 64)
    g2 = sigmoid((T/145) @ w_att + b_att)              # (1000, 64)
    smi_v = mean_i U[i]*(0.5+g1[i]); pro_v = mean_j pro[j]*(0.5+g2[j])
    out = MLP(concat([smi_v, pro_v]))                  # (2,)

On-chip layout is transposed (D on partitions). The pairwise phase
splits the j axis between the Scalar engine (activation with per-
partition bias + fused row-sum) and the Vector engine (tensor_scalar
add+max with fused row-sum); the Tensor engine accumulates T in PSUM
via identity matmuls.
"""

import numpy as np

import concourse.bacc as bacc
import concourse.mybir as mybir
from concourse import masks, tile
from concourse.tile import add_dep_helper
from concourse.bass_utils import run_bass_kernel_spmd

F32 = mybir.dt.float32
BF16 = mybir.dt.bfloat16
AF = mybir.ActivationFunctionType
ALU = mybir.AluOpType

B, NS, NA, NP, D = 8, 100, 45, 1000, 64
ND = NS + NA          # 145
NT = (ND + 1) // 2    # 73 pairwise iterations, 2 i-values each
H1, H2, H3, HO = 1024, 1024, 512, 2

# j-axis split between engines in the pairwise loop.
J_ACT = 488                  # scalar engine takes V2[:, :J_ACT]
J_DVE = NP - J_ACT           # vector engine takes the rest (512)

NEG = -1.0e30


def _build(dbg=False):
    nc = bacc.Bacc("TRN2", target_bir_lowering=False, debug=False)

    smi = nc.dram_tensor("smi", (NS, D), F32, kind="ExternalInput").ap()
    pro = nc.dram_tensor("pro", (NP, D), F32, kind="ExternalInput").ap()
    gat = nc.dram_tensor("gat", (NA, D), F32, kind="ExternalInput").ap()
    w_att = nc.dram_tensor("w_att", (D, D), F32, kind="ExternalInput").ap()
    b_att = nc.dram_tensor("b_att", (D,), F32, kind="ExternalInput").ap()
    w1 = nc.dram_tensor("w1", (2 * D, H1), BF16, kind="ExternalInput").ap()
    b1 = nc.dram_tensor("b1", (H1,), F32, kind="ExternalInput").ap()
    w2 = nc.dram_tensor("w2", (H1, H2), BF16, kind="ExternalInput").ap()
    b2 = nc.dram_tensor("b2", (H2,), F32, kind="ExternalInput").ap()
    w3 = nc.dram_tensor("w3", (H2, H3), BF16, kind="ExternalInput").ap()
    b3 = nc.dram_tensor("b3", (H3,), F32, kind="ExternalInput").ap()
    w4 = nc.dram_tensor("w4", (H3, HO), BF16, kind="ExternalInput").ap()
    b4 = nc.dram_tensor("b4", (HO,), F32, kind="ExternalInput").ap()
    out = nc.dram_tensor("out", (HO,), F32, kind="ExternalOutput").ap()

    dbg_out = {}
    if dbg:
        for name, shape in [
            ("d_U2", (128, 2 * NT)), ("d_PT", (D, NP)),
            ("d_Sa", (128, NT)), ("d_Sd", (128, NT)), ("d_Tsb", (128, NP)),
            ("d_G1", (D, 2 * NT)), ("d_G2", (D, NP)),
            ("d_sv", (D, 1)), ("d_pv", (D, 1)),
        ]:
            dbg_out[name] = nc.dram_tensor(name, shape, F32, kind="ExternalOutput").ap()
    with tile.TileContext(nc) as tc:
        _body(nc, tc, smi, pro, gat, w_att, b_att,
              w1, b1, w2, b2, w3, b3, w4, b4, out, dbg_out)
    nc.compile()
    return nc


def _body(nc, tc, smi, pro, gat, w_att, b_att,
          w1, b1, w2, b2, w3, b3, w4, b4, out, dbg_out=()):
    with (
        tc.tile_pool(name="const", bufs=1) as cp,
        tc.tile_pool(name="ra", bufs=3) as rap,
        tc.tile_pool(name="rd", bufs=3) as rdp,
        tc.tile_pool(name="pst", bufs=1, space="PSUM") as pst,
        tc.tile_pool(name="psw", bufs=2, space="PSUM") as psw,
        tc.tile_pool(name="psh", bufs=2, space="PSUM") as psh,
    ):
        # ---------------- phase-A input DMAs (issued first) -----------
        ident = cp.tile([128, 128], F32)
        masks.make_identity(nc, ident[:])
        identb = cp.tile([128, 128], BF16)
        masks.make_identity(nc, identb[:])
        PRO = cp.tile([125, 8 * D], F32)
        pro_r = pro.rearrange("(n p) d -> p n d", p=125)
        pro_dmas = []
        for c in range(4):
            pro_dmas.append(nc.sync.dma_start(
                PRO[:, 2 * c * D:2 * (c + 1) * D].rearrange(
                    "p (n d) -> p n d", n=2),
                pro_r[:, 2 * c:2 * c + 2, :]))
        SMI = cp.tile([NS, D], F32)
        nc.sync.dma_start(SMI[:], smi[:])
        GA2 = cp.tile([NA, 128], F32)
        nc.sync.dma_start(GA2[:, 0:D], gat[:])
        nc.sync.dma_start(GA2[:, D:128], gat[:])

        # w_att in the three stacked forms the matmul tricks need
        wdup = cp.tile([D, 128], F32)      # [w | w]   -> duplicated M
        nc.sync.dma_start(wdup[:, 0:D], w_att[:])
        nc.sync.dma_start(wdup[:, D:2 * D], w_att[:])
        wstk = cp.tile([128, D], BF16)     # [w ; w]   -> K-stacked (fold)
        nc.gpsimd.dma_start(wstk[0:D, :], w_att[:])
        nc.gpsimd.dma_start(wstk[D:128, :], w_att[:])
        zdg = cp.tile([128, 128], F32)     # blockdiag(w, w)
        nc.vector.memset(zdg[:], 0.0)
        nc.sync.dma_start(zdg[0:D, 0:D], w_att[:])
        nc.sync.dma_start(zdg[D:128, D:128], w_att[:])
        bdup = cp.tile([128, 1], F32)      # [b_att ; b_att]
        b_col = b_att.rearrange("(d a) -> d a", a=1)
        nc.sync.dma_start(bdup[0:D, :], b_col)
        nc.sync.dma_start(bdup[D:128, :], b_col)

        # ---------------- weight / constant DMAs ----------------------
        wdmas = []
        W1a = cp.tile([D, H1], BF16)
        W1b = cp.tile([D, H1], BF16)
        wdmas.append(nc.gpsimd.dma_start(W1a[:], w1[0:D, :]))
        wdmas.append(nc.gpsimd.dma_start(W1b[:], w1[D:2 * D, :]))
        W2 = cp.tile([128, 8, H2], BF16)
        w2r = w2.rearrange("(c p) n -> p c n", p=128)
        for c in range(4):
            wdmas.append(nc.gpsimd.dma_start(W2[:, 2 * c:2 * c + 2, :],
                                             w2r[:, 2 * c:2 * c + 2, :]))
        W3 = cp.tile([128, 8, H3], BF16)
        w3r = w3.rearrange("(c p) n -> p c n", p=128)
        for c in range(2):
            wdmas.append(nc.gpsimd.dma_start(W3[:, 4 * c:4 * c + 4, :],
                                             w3r[:, 4 * c:4 * c + 4, :]))
        W4 = cp.tile([128, 4, HO], BF16)
        wdmas.append(nc.gpsimd.dma_start(W4[:], w4.rearrange("(c p) n -> p c n", p=128)))
        # weights are needed only by the MLP tail; keep them off the wire
        # until the latency-critical pro/smi/gat inputs have landed
        for wd in wdmas:
            add_dep_helper(wd.ins, pro_dmas[-1].ins, sync=True,
                           reason="delay weight DMA behind critical inputs")
        B1sb = cp.tile([128, 8], F32)
        nc.gpsimd.dma_start(B1sb[:], b1.rearrange("(c p) -> p c", p=128))
        B2sb = cp.tile([128, 8], F32)
        nc.gpsimd.dma_start(B2sb[:], b2.rearrange("(c p) -> p c", p=128))
        B3sb = cp.tile([128, 4], F32)
        nc.gpsimd.dma_start(B3sb[:], b3.rearrange("(c p) -> p c", p=128))
        B4sb = cp.tile([HO, 1], F32)
        nc.gpsimd.dma_start(B4sb[:], b4.rearrange("(d a) -> d a", a=1))


        # ---------------- phase A: transposes + projections -----------
        # pro (1000,64) loaded as (125, 8*64); chunk c covers rows
        # c*125 .. c*125+124.
        # U2 (128, 146): lower half = U^T columns 0..144, upper half =
        # U^T columns shifted by one; column 145 (and upper 144) are the
        # -1e30 pad so the pair (144,145) contributes relu()=0 for the
        # dummy index.
        U2 = cp.tile([128, 2 * NT], F32)
        nc.vector.memset(U2[:], NEG)
        ps = psw.tile([D, NS], F32, tag="ps")
        nc.tensor.transpose(ps[:], SMI[:], ident[0:NS, 0:NS])
        SMT = cp.tile([D, NS], F32)
        nc.scalar.copy(SMT[:], ps[:])
        ps = psw.tile([128, NS], F32, tag="ps")
        nc.tensor.matmul(ps[:], wdup[:], SMT[:])
        nc.scalar.activation(U2[0:D, 0:NS], ps[0:D, :],
                             AF.Identity, bias=bdup[0:D, 0:1])
        nc.scalar.activation(U2[D:128, 0:NS - 1], ps[D:128, 1:NS],
                             AF.Identity, bias=bdup[D:128, 0:1])
        ps = psw.tile([128, NA], F32, tag="ps")
        nc.tensor.matmul(ps[:], GA2[:], ident[0:NA, 0:NA])
        nc.scalar.copy(U2[0:D, NS:ND], ps[0:D, :])
        nc.scalar.copy(U2[D:128, NS - 1:ND - 1], ps[D:128, :])

        PT = cp.tile([D, NP], F32)         # pro^T
        V2 = cp.tile([128, NP], BF16)      # [pro_att^T ; pro_att^T]
        for c in range(8):
            cc = slice(c * 125, (c + 1) * 125)
            ps = psw.tile([D, 125], F32, tag="ps")
            nc.tensor.transpose(ps[:], PRO[:, c * D:(c + 1) * D], ident[0:125, 0:125])
            if c % 2 == 0:
                nc.scalar.copy(PT[:, cc], ps[:])
            else:
                nc.vector.tensor_copy(PT[:, cc], ps[:])
            if c % 4 == 3:
                h = c // 4
                pv2 = psw.tile([128, 500], F32, tag="pv2")
                nc.tensor.matmul(pv2[:], wdup[:], PT[:, 500 * h:500 * (h + 1)])
                if h == 0:
                    nc.scalar.activation(V2[:, 0:500], pv2[:],
                                         AF.Identity, bias=bdup[:, 0:1])
                else:
                    nc.vector.tensor_scalar(V2[:, 500:1000], pv2[:],
                                            bdup[:, 0:1], None, ALU.add)

        # ---------------- phase B: pairwise relu-sum loop -------------
        TAIL = 6                           # last iterations: DVE does both
        Sa = cp.tile([128, NT], F32)       # row-sums from ACT slice
        nc.vector.memset(Sa[:, NT - TAIL:NT], 0.0)
        Sa2 = cp.tile([128, TAIL], F32)    # ACT-slice sums of tail iters
        Sd = cp.tile([128, NT], F32)       # row-sums from DVE slice
        Zz = cp.tile([128, J_DVE], BF16)   # zeros for the DVE relu clamp
        nc.vector.memset(Zz[:], 0.0)
        TA = pst.tile([128, J_ACT], F32, tag="ta")
        TD = pst.tile([128, J_DVE], F32, tag="td")
        for t in range(NT):
            u_col = U2[:, 2 * t:2 * t + 1]
            Ra = rap.tile([128, J_ACT], BF16, tag="ra")
            Rd = rdp.tile([128, J_DVE], BF16, tag="rd")
            if t < NT - TAIL:
                nc.scalar.activation(Ra[:], V2[:, 0:J_ACT], AF.Relu,
                                     bias=u_col, accum_out=Sa[:, t:t + 1])
            else:
                nc.vector.scalar_tensor_tensor(
                    Ra[:], V2[:, 0:J_ACT], u_col, Zz[:, 0:J_ACT],
                    ALU.add, ALU.max,
                    accum_out=Sa2[:, t - (NT - TAIL):t - (NT - TAIL) + 1])
            nc.vector.scalar_tensor_tensor(Rd[:], V2[:, J_ACT:NP], u_col,
                                           Zz[:], ALU.add, ALU.max,
                                           accum_out=Sd[:, t:t + 1])
            st, sp = (t == 0), (t == NT - 1)
            nc.tensor.matmul(TA[:], identb[:], Ra[:], start=st, stop=sp)
            nc.tensor.matmul(TD[:], identb[:], Rd[:], start=st, stop=sp)

        # ---------------- phase C: gates + pooled vectors -------------
        # T (fold of upper/lower halves) -> g2, via K=128 matmul with
        # the K-stacked w_att.
        Tsb = cp.tile([128, NP], BF16)
        G2 = cp.tile([D, NP], F32)
        nc.scalar.copy(Tsb[:, J_ACT:J_ACT + 250], TD[:, 0:250])
        nc.vector.tensor_copy(Tsb[:, J_ACT + 250:NP], TD[:, 250:J_DVE])
        nc.scalar.copy(Tsb[:, 0:250], TA[:, 0:250])
        nc.vector.tensor_copy(Tsb[:, 250:J_ACT], TA[:, 250:J_ACT])
        for c in (2, 3, 0, 1):
            cc = slice(250 * c, 250 * (c + 1))
            ps = psw.tile([D, 250], F32, tag="ps")
            nc.tensor.matmul(ps[:], wstk[:], Tsb[:, cc])
            nc.scalar.activation(G2[:, cc], ps[:], AF.Sigmoid,
                                 bias=bdup[0:D, 0:1], scale=1.0 / ND)

        # S2 = Sa + Sd; g1 halves via blockdiag(w,w) matmuls.
        S2 = cp.tile([128, NT], F32)
        nc.vector.tensor_tensor(S2[:], Sa[:], Sd[:], ALU.add)
        nc.vector.tensor_tensor(S2[:, NT - TAIL:NT], S2[:, NT - TAIL:NT],
                                Sa2[:], ALU.add)
        psm = psw.tile([D, 2 * NT], F32, tag="ps")
        nc.tensor.matmul(psm[:, 0:NT], zdg[:, 0:D], S2[:])
        nc.tensor.matmul(psm[:, NT:2 * NT], zdg[:, D:128], S2[:])
        G1 = cp.tile([D, 2 * NT], F32)
        nc.scalar.activation(G1[:], psm[:], AF.Sigmoid,
                             bias=bdup[0:D, 0:1], scale=1.0 / NP)

        # pooled vectors
        pe = cp.tile([D, NT], F32)
        po = cp.tile([D, NT - 1], F32)
        pp = cp.tile([D, NP], F32)
        se = cp.tile([D, 1], F32)
        so = cp.tile([D, 1], F32)
        sp_ = cp.tile([D, 1], F32)
        nc.vector.scalar_tensor_tensor(pe[:], G1[:, 0:NT], 0.5,
                                       U2[0:D, 0:2 * NT - 1:2],
                                       ALU.add, ALU.mult, accum_out=se[:])
        nc.vector.scalar_tensor_tensor(po[:], G1[:, NT:2 * NT - 1], 0.5,
                                       U2[0:D, 1:2 * NT - 2:2],
                                       ALU.add, ALU.mult, accum_out=so[:])
        sp4 = cp.tile([D, 4], F32)
        for c in (2, 3, 0, 1):
            cc = slice(250 * c, 250 * (c + 1))
            nc.vector.scalar_tensor_tensor(pp[:, cc], G2[:, cc], 0.5, PT[:, cc],
                                           ALU.add, ALU.mult,
                                           accum_out=sp4[:, c:c + 1])
        nc.vector.tensor_reduce(sp_[:], sp4[:], mybir.AxisListType.X, ALU.add)
        sv = cp.tile([D, 1], F32)
        nc.vector.tensor_tensor(sv[:], se[:], so[:], ALU.add)
        smi_v = cp.tile([D, 1], F32)
        nc.scalar.mul(smi_v[:], sv[:], 1.0 / ND)
        pro_v = cp.tile([D, 1], F32)
        nc.scalar.mul(pro_v[:], sp_[:], 1.0 / NP)

        # ---------------- phase D: MLP head (bf16 weights) ------------
        smi_vb = cp.tile([D, 1], BF16)
        nc.scalar.copy(smi_vb[:], smi_v[:])
        pro_vb = cp.tile([D, 1], BF16)
        nc.scalar.copy(pro_vb[:], pro_v[:])

        ph1 = psh.tile([128, 8], F32, tag="h")
        for m in range(8):
            mm = slice(128 * m, 128 * (m + 1))
            nc.tensor.matmul(ph1[:, m:m + 1], W1a[:, mm], smi_vb[:],
                             start=True, stop=False)
            nc.tensor.matmul(ph1[:, m:m + 1], W1b[:, mm], pro_vb[:],
                             start=False, stop=True)
        nc.vector.tensor_tensor(ph1[:], ph1[:], B1sb[:], ALU.add)
        Ht1 = cp.tile([128, 8], BF16)
        nc.scalar.activation(Ht1[:], ph1[:], AF.Relu)

        ph2 = psh.tile([128, 8], F32, tag="h")
        for m in range(8):
            mm = slice(128 * m, 128 * (m + 1))
            for c in range(8):
                nc.tensor.matmul(ph2[:, m:m + 1], W2[:, c, mm], Ht1[:, c:c + 1],
                                 start=(c == 0), stop=(c == 7))
        nc.vector.tensor_tensor(ph2[:], ph2[:], B2sb[:], ALU.add)
        Ht2 = cp.tile([128, 8], BF16)
        nc.scalar.activation(Ht2[:], ph2[:], AF.Relu)

        ph3 = psh.tile([128, 4], F32, tag="h")
        for m in range(4):
            mm = slice(128 * m, 128 * (m + 1))
            for c in range(8):
                nc.tensor.matmul(ph3[:, m:m + 1], W3[:, c, mm], Ht2[:, c:c + 1],
                                 start=(c == 0), stop=(c == 7))
        nc.vector.tensor_tensor(ph3[:], ph3[:], B3sb[:], ALU.add)
        Ht3 = cp.tile([128, 4], BF16)
        nc.scalar.activation(Ht3[:], ph3[:], AF.Relu)

        ph4 = psh.tile([HO, 1], F32, tag="h")
        for c in range(4):
            nc.tensor.matmul(ph4[:], W4[:, c, :], Ht3[:, c:c + 1],
                             start=(c == 0), stop=(c == 3))
        nc.vector.tensor_tensor(ph4[:], ph4[:], B4sb[:], ALU.add)
        osb = cp.tile([HO, 1], F32)
        nc.scalar.copy(osb[:], ph4[:])
        nc.sync.dma_start(out.rearrange("(a b) -> a b", b=1), osb[:])

        if dbg_out:
            for name, t in [("d_U2", U2), ("d_PT", PT),
                            ("d_Sa", Sa), ("d_Sd", Sd), ("d_Tsb", Tsb),
                            ("d_G1", G1), ("d_G2", G2),
                            ("d_sv", smi_v), ("d_pv", pro_v)]:
                nc.sync.dma_start(dbg_out[name], t[:])


_NC = None


def kernel(smi_tf, pro_tf, drug_gat, w_att, b_att,
           w1, b1, w2, b2, w3, b3, w4, b4):
    global _NC
    if _NC is None:
        _NC = _build()
    import ml_dtypes
    f32 = lambda a: np.ascontiguousarray(np.asarray(a), dtype=np.float32)
    bf16 = lambda a: np.ascontiguousarray(np.asarray(a), dtype=ml_dtypes.bfloat16)
    shared = {
        "w_att": f32(w_att), "b_att": f32(b_att),
        "w1": bf16(w1), "b1": f32(b1), "w2": bf16(w2), "b2": f32(b2),
        "w3": bf16(w3), "b3": f32(b3), "w4": bf16(w4), "b4": f32(b4),
    }
    in_maps = [
        {"smi": f32(smi_tf[b]), "pro": f32(pro_tf[b]),
         "gat": f32(drug_gat[b]), **shared}
        for b in range(B)
    ]
    res = run_bass_kernel_spmd(_NC, in_maps, core_ids=list(range(B)))
    return np.stack([res.results[b]["out"] for b in range(B)], axis=0)

